# revision 4
# baseline (speedup 1.0000x reference)
"""Trainium2 Bass kernel: GNN message passing (embed gather -> ragged GRU -> LN -> FC).

Sharding: destination nodes are sorted by mailbox length and dealt across the
8 cores so every core gets an equal mix of lengths.  Within a core, nodes of
equal length are processed together, so each group runs exactly (l-1) GRU
steps with no masking.  The embedding table (fp16) and the small GRU/LN/FC
parameters are replicated; the src-feature gather runs on-device via
indirect DMA from the fp16 table.  The GRU runs in transposed space
([D, nodes]) so no per-step transposes are needed; LayerNorm statistics are
computed with ones-matmuls and applied via outer-product scale/offset.
"""

import sys

sys.path.insert(0, "/opt/trn_rl_repo")

import numpy as np

import concourse.bass as bass
import concourse.bacc as bacc
import concourse.tile as tile
from concourse import mybir
from concourse.bass_utils import run_bass_kernel_spmd
from concourse.masks import make_identity

P = 128
D = 128
V = 50000
C = 104
L_MAX = 8
N_DST = 100000
N_CORES = 8
UNIT = 512  # nodes per processing unit (PSUM bank = 512 f32)

_cache = {}


def _plan():
    """Static plan: per-length padded per-core row counts and unit sizes.

    lengths are data, but the unit structure only depends on per-core group
    sizes, which we equalize by padding to max across cores; computed at
    runtime in kernel() from the actual lengths.
    """


def _unit_sizes(m_pad):
    out = []
    left = m_pad
    while left >= UNIT:
        out.append(UNIT)
        left -= UNIT
    if left:
        out.append(left)
    return out


def _build(group_rows):
    """group_rows: dict l -> padded per-core rows M_l (multiple of 128).

    Returns (nc, ntot).
    """
    ntot = sum(group_rows.values())
    nc = bacc.Bacc("TRN2", target_bir_lowering=False, debug=False,
                   num_devices=N_CORES)

    tid_d = nc.dram_tensor("tid", [ntot, L_MAX], mybir.dt.int32,
                           kind="ExternalInput")
    tab_d = nc.dram_tensor("tab", [V, D], mybir.dt.float16,
                           kind="ExternalInput")
    wt_d = nc.dram_tensor("wt", [D, 6 * D], mybir.dt.float16,
                          kind="ExternalInput")
    bias_d = nc.dram_tensor("bias", [D, 4], mybir.dt.float32,
                            kind="ExternalInput")
    gb_d = nc.dram_tensor("gb", [1, 2 * D], mybir.dt.float16,
                          kind="ExternalInput")
    fcw_d = nc.dram_tensor("fcw", [D, C], mybir.dt.float16,
                           kind="ExternalInput")
    fcb_d = nc.dram_tensor("fcb", [P, C], mybir.dt.float32,
                           kind="ExternalInput")
    out_d = nc.dram_tensor("out", [ntot, C], mybir.dt.float32,
                           kind="ExternalOutput")

    f16 = mybir.dt.float16
    f32 = mybir.dt.float32

    with tile.TileContext(nc) as tc:
        with (
            tc.tile_pool(name="const", bufs=1) as constp,
            tc.tile_pool(name="tid", bufs=4) as tidp,
            tc.tile_pool(name="msg", bufs=2) as msgp,
            tc.tile_pool(name="msgT", bufs=2) as msgTp,
            tc.tile_pool(name="h", bufs=3) as hp,
            tc.tile_pool(name="gate", bufs=2) as gatep,
            tc.tile_pool(name="stat", bufs=4) as statp,
            tc.tile_pool(name="olog", bufs=4) as ologp,
            tc.tile_pool(name="ptr", bufs=1, space="PSUM") as ptrp,
            tc.tile_pool(name="pgru", bufs=1, space="PSUM") as pgrup,
            tc.tile_pool(name="pst", bufs=1, space="PSUM") as pstp,
            tc.tile_pool(name="pout", bufs=1, space="PSUM") as poutp,
        ):
            # --- constants ---
            wt = constp.tile([D, 6 * D], f16)
            nc.sync.dma_start(out=wt[:], in_=wt_d[:])
            bias = constp.tile([D, 4], f32)
            nc.sync.dma_start(out=bias[:], in_=bias_d[:])
            gbr = constp.tile([1, 2 * D], f16)
            nc.sync.dma_start(out=gbr[:], in_=gb_d[:])
            fcw = constp.tile([D, C], f16)
            nc.sync.dma_start(out=fcw[:], in_=fcw_d[:])
            fcb = constp.tile([P, C], f32)
            nc.sync.dma_start(out=fcb[:], in_=fcb_d[:])
            ident = constp.tile([P, P], f16)
            make_identity(nc, ident[:])
            onesc = constp.tile([D, 1], f16)
            nc.vector.memset(onesc[:], 1.0 / 128.0)
            onesr = constp.tile([1, UNIT], f16)
            nc.vector.memset(onesr[:], 1.0)

            b_r = bias[:, 0:1]
            b_z = bias[:, 1:2]
            b_ihn = bias[:, 2:3]
            b_hhn = bias[:, 3:4]
            g_row = gbr[0:1, 0:D]
            bl_row = gbr[0:1, D:2 * D]

            # W_ih.T gates at cols [0:384]; W_hh.T gates at [384:768]
            w_ir = wt[:, 0 * D:1 * D]
            w_iz = wt[:, 1 * D:2 * D]
            w_in = wt[:, 2 * D:3 * D]
            w_hr = wt[:, 3 * D:4 * D]
            w_hz = wt[:, 4 * D:5 * D]
            w_hn = wt[:, 5 * D:6 * D]

            # build unit list interleaved across lengths for engine balance
            units = []
            base = 0
            for l in range(1, L_MAX + 1):
                b = base
                for n in _unit_sizes(group_rows[l]):
                    units.append((l, b, n))
                    b += n
                base += group_rows[l]
            units.sort(key=lambda u: (u[1] % UNIT, -u[0]))

            add = mybir.AluOpType.add
            sub = mybir.AluOpType.subtract
            mult = mybir.AluOpType.mult
            AF = mybir.ActivationFunctionType

            for (l, rbase, n) in units:
                nb = n // P
                # -- gather all l message columns (natural layout), per block
                msg = msgp.tile([P, L_MAX, UNIT], f16, tag="msg")
                tidts = []
                for b in range(nb):
                    tt_ = tidp.tile([P, L_MAX], mybir.dt.int32, tag="tid")
                    nc.sync.dma_start(
                        out=tt_[:],
                        in_=tid_d[rbase + b * P: rbase + (b + 1) * P, :])
                    tidts.append(tt_)
                for t in range(l):
                    for b in range(nb):
                        nc.gpsimd.indirect_dma_start(
                            out=msg[:, t, b * P:(b + 1) * P],
                            out_offset=None,
                            in_=tab_d[:],
                            in_offset=bass.IndirectOffsetOnAxis(
                                ap=tidts[b][:, t:t + 1], axis=0),
                        )
                # -- transpose each column into msgT
                msgT = msgTp.tile([P, L_MAX, UNIT], f16, tag="msgT")
                for t in range(l):
                    ptr = ptrp.tile([P, UNIT], f16, space="PSUM", tag="ptr")
                    for b in range(nb):
                        nc.tensor.transpose(
                            out=ptr[:, b * P:(b + 1) * P],
                            in_=msg[:, t, b * P:(b + 1) * P],
                            identity=ident[:])
                    nc.scalar.copy(out=msgT[:, t, :n], in_=ptr[:, :n])

                hT = msgT[:, l - 1, :n]
                # -- GRU steps
                for t in range(l - 1):
                    xT = msgT[:, t, :n]
                    ps_r = pgrup.tile([P, UNIT], f32, space="PSUM", tag="ps_r")
                    nc.tensor.matmul(out=ps_r[:, :n], lhsT=w_ir, rhs=xT,
                                     start=True, stop=False)
                    nc.tensor.matmul(out=ps_r[:, :n], lhsT=w_hr, rhs=hT,
                                     start=False, stop=True)
                    ps_z = pgrup.tile([P, UNIT], f32, space="PSUM", tag="ps_z")
                    nc.tensor.matmul(out=ps_z[:, :n], lhsT=w_iz, rhs=xT,
                                     start=True, stop=False)
                    nc.tensor.matmul(out=ps_z[:, :n], lhsT=w_hz, rhs=hT,
                                     start=False, stop=True)
                    ps_in = pgrup.tile([P, UNIT], f32, space="PSUM",
                                       tag="ps_in")
                    nc.tensor.matmul(out=ps_in[:, :n], lhsT=w_in, rhs=xT,
                                     start=True, stop=True)
                    ps_hn = pgrup.tile([P, UNIT], f32, space="PSUM",
                                       tag="ps_hn")
                    nc.tensor.matmul(out=ps_hn[:, :n], lhsT=w_hn, rhs=hT,
                                     start=True, stop=True)

                    r = gatep.tile([P, UNIT], f16, tag="r")
                    nc.scalar.activation(out=r[:, :n], in_=ps_r[:, :n],
                                         func=AF.Sigmoid, bias=b_r)
                    z = gatep.tile([P, UNIT], f16, tag="z")
                    nc.scalar.activation(out=z[:, :n], in_=ps_z[:, :n],
                                         func=AF.Sigmoid, bias=b_z)
                    rhn = gatep.tile([P, UNIT], f16, tag="rhn")
                    nc.vector.scalar_tensor_tensor(
                        out=rhn[:, :n], in0=ps_hn[:, :n], scalar=b_hhn,
                        in1=r[:, :n], op0=add, op1=mult)
                    npre = gatep.tile([P, UNIT], f16, tag="npre")
                    nc.vector.tensor_tensor(out=npre[:, :n], in0=rhn[:, :n],
                                            in1=ps_in[:, :n], op=add)
                    nt = gatep.tile([P, UNIT], f16, tag="nt")
                    nc.scalar.activation(out=nt[:, :n], in_=npre[:, :n],
                                         func=AF.Tanh, bias=b_ihn)
                    dt_ = gatep.tile([P, UNIT], f16, tag="dt")
                    nc.vector.tensor_tensor(out=dt_[:, :n], in0=hT,
                                            in1=nt[:, :n], op=sub)
                    zd = gatep.tile([P, UNIT], f16, tag="zd")
                    nc.vector.tensor_tensor(out=zd[:, :n], in0=z[:, :n],
                                            in1=dt_[:, :n], op=mult)
                    hnew = hp.tile([P, UNIT], f16, tag="h")
                    nc.vector.tensor_tensor(out=hnew[:, :n], in0=nt[:, :n],
                                            in1=zd[:, :n], op=add)
                    hT = hnew[:, :n]

                if l > 1:
                    # -- LayerNorm in transposed space
                    sq = gatep.tile([P, UNIT], f16, tag="sq")
                    nc.scalar.activation(out=sq[:, :n], in_=hT,
                                         func=AF.Square)
                    ps_st = pstp.tile([33, UNIT], f32, space="PSUM",
                                      tag="ps_st")
                    nc.tensor.matmul(out=ps_st[0:1, :n], lhsT=onesc, rhs=hT,
                                     start=True, stop=True)
                    nc.tensor.matmul(out=ps_st[32:33, :n], lhsT=onesc,
                                     rhs=sq[:, :n], start=True, stop=True)
                    ps_mu = ps_st[0:1, :]
                    ps_msq = ps_st[32:33, :]
                    mu = statp.tile([1, UNIT], f32, tag="mu")
                    nc.scalar.copy(out=mu[:, :n], in_=ps_mu[:, :n])
                    mm_ = statp.tile([1, UNIT], f32, tag="mm")
                    nc.vector.tensor_tensor(out=mm_[:, :n], in0=mu[:, :n],
                                            in1=mu[:, :n], op=mult)
                    v = statp.tile([1, UNIT], f32, tag="v")
                    nc.vector.scalar_tensor_tensor(
                        out=v[:, :n], in0=ps_msq[:, :n], scalar=1e-5,
                        in1=mm_[:, :n], op0=add, op1=sub)
                    rec = statp.tile([1, UNIT], f32, tag="rec")
                    nc.vector.reciprocal(out=rec[:, :n], in_=v[:, :n])
                    rstd = statp.tile([1, UNIT], f16, tag="rstd")
                    nc.scalar.activation(out=rstd[:, :n], in_=rec[:, :n],
                                         func=AF.Sqrt)
                    cneg = statp.tile([1, UNIT], f16, tag="cneg")
                    nc.vector.scalar_tensor_tensor(
                        out=cneg[:, :n], in0=mu[:, :n], scalar=-1.0,
                        in1=rstd[:, :n], op0=mult, op1=mult)
                    ps_a = poutp.tile([P, UNIT], f32, space="PSUM",
                                      tag="ac")
                    nc.tensor.matmul(out=ps_a[:, :n], lhsT=g_row,
                                     rhs=rstd[:, :n], start=True, stop=True)
                    t1 = gatep.tile([P, UNIT], f16, tag="t1")
                    nc.vector.tensor_tensor(out=t1[:, :n], in0=hT,
                                            in1=ps_a[:, :n], op=mult)
                    ps_c = poutp.tile([P, UNIT], f32, space="PSUM",
                                      tag="ac")
                    nc.tensor.matmul(out=ps_c[:, :n], lhsT=g_row,
                                     rhs=cneg[:, :n], start=True, stop=False)
                    nc.tensor.matmul(out=ps_c[:, :n], lhsT=bl_row,
                                     rhs=onesr[:, :n], start=False, stop=True)
                    lnT = hp.tile([P, UNIT], f16, tag="lnT")
                    nc.vector.tensor_tensor(out=lnT[:, :n], in0=t1[:, :n],
                                            in1=ps_c[:, :n], op=add)
                    outT = lnT[:, :n]
                else:
                    outT = hT

                # -- FC head per 128-node block
                for b in range(nb):
                    ps_fc = poutp.tile([P, C], f32, space="PSUM", tag="ps_fc")
                    nc.tensor.matmul(out=ps_fc[:],
                                     lhsT=outT[:, b * P:(b + 1) * P],
                                     rhs=fcw[:], start=True, stop=True)
                    lg = ologp.tile([P, C], f32, tag="lg")
                    nc.vector.tensor_tensor(out=lg[:], in0=ps_fc[:],
                                            in1=fcb[:], op=add)
                    nc.sync.dma_start(
                        out=out_d[rbase + b * P: rbase + (b + 1) * P, :],
                        in_=lg[:])

    nc.compile()
    return nc, ntot


def kernel(token_id, mailbox_idx, lengths, embed, W_ih, W_hh, b_ih, b_hh,
           ln_g, ln_b, fc_w, fc_b):
    token_id = np.asarray(token_id)
    mailbox_idx = np.asarray(mailbox_idx)
    lengths = np.asarray(lengths)
    embed = np.asarray(embed, dtype=np.float32)
    W_ih = np.asarray(W_ih, dtype=np.float32)
    W_hh = np.asarray(W_hh, dtype=np.float32)
    b_ih = np.asarray(b_ih, dtype=np.float32)
    b_hh = np.asarray(b_hh, dtype=np.float32)
    ln_g = np.asarray(ln_g, dtype=np.float32)
    ln_b = np.asarray(ln_b, dtype=np.float32)
    fc_w = np.asarray(fc_w, dtype=np.float32)
    fc_b = np.asarray(fc_b, dtype=np.float32)

    n_dst = mailbox_idx.shape[0]
    # src tokens per mailbox slot (index composition on host; feature gather
    # stays on device)
    tid_full = token_id[mailbox_idx].astype(np.int32)  # [n_dst, L_MAX]

    # group nodes by length, deal each group across cores
    ids_by = {l: np.where(lengths == l)[0] for l in range(1, L_MAX + 1)}
    core_ids = {}  # (core, l) -> node ids
    group_rows = {}
    for l in range(1, L_MAX + 1):
        m = 0
        for k in range(N_CORES):
            ids = ids_by[l][k::N_CORES]
            core_ids[(k, l)] = ids
            m = max(m, len(ids))
        group_rows[l] = ((m + P - 1) // P) * P

    key = tuple(sorted(group_rows.items()))
    if key not in _cache:
        _cache[key] = _build(group_rows)
    nc, ntot = _cache[key]

    # per-core host-side shards
    tab16 = embed.astype(np.float16)
    wt = np.concatenate([W_ih.T, W_hh.T], axis=1).astype(np.float16)
    bsum = (b_ih + b_hh)
    bias4 = np.stack([bsum[0:D], bsum[D:2 * D], b_ih[2 * D:3 * D],
                      b_hh[2 * D:3 * D]], axis=1).astype(np.float32)
    gb = np.concatenate([ln_g, ln_b]).astype(np.float16)[None, :]
    fcw = fc_w.T.astype(np.float16)
    fcb = np.broadcast_to(fc_b[None, :], (P, C)).astype(np.float32).copy()

    in_maps = []
    for k in range(N_CORES):
        tid_k = np.zeros((ntot, L_MAX), np.int32)
        base = 0
        for l in range(1, L_MAX + 1):
            ids = core_ids[(k, l)]
            tid_k[base:base + len(ids)] = tid_full[ids]
            base += group_rows[l]
        in_maps.append({
            "tid": tid_k, "tab": tab16, "wt": wt, "bias": bias4,
            "gb": gb, "fcw": fcw, "fcb": fcb,
        })

    res = run_bass_kernel_spmd(nc, in_maps, core_ids=list(range(N_CORES)))

    out = np.zeros((n_dst, C), np.float32)
    for k in range(N_CORES):
        ok = res.results[k]["out"]
        base = 0
        for l in range(1, L_MAX + 1):
            ids = core_ids[(k, l)]
            out[ids] = ok[base:base + len(ids)]
            base += group_rows[l]
    return out
